# revision 9
# baseline (speedup 1.0000x reference)
"""Trainium2 Bass kernel for single-head MHA (B=32, G=1024, D=256),
data-parallel over batch across 8 NeuronCores.

v2: bf16 matmul operands (1 cyc/row PE rate, half-size PSUM drains),
masked-key compaction (the host stable-sorts each batch's rows so unmasked
keys come first; only NKT=5 of 8 key tiles are computed — the per-partition
exp bias of -100 zeroes the masked tail inside tile NKT-1, and tiles
NKT..7 are entirely masked so they are skipped), one wide exp per key tile
([128,1024] across a 2-bank PSUM tile), and engine-balanced PSUM drains
(QT/HT/l on ScalarE, rest on VectorE).

Per-core per-batch algorithm (no GxG transposes anywhere):
  dT   = data_b^T              [D, G]  bf16 (PE transposes of dn16)
  QT   = NT @ dT               [D, G]  bf16 (NT = bf16(Wq^T Wk), folded QK)
  ST   = dT_k^T @ QT  (= S^T)  [128, G] f32 per key tile kt < NKT
  PT   = exp(NORM*ST + bias_k) bias_k = -100*mask[k] per-partition on ACT
  HT  += V_kt^T @ PT           [D, G]  V = dn16 (value proj folded into PTO)
  l    = ones^T @ PT           [1, G]
  F    = HT^T @ PTO            [G, D]  PTO = bf16(Wv^T Wo^T)
  out  = F * (1/l)[q] + b_out  (scalar_tensor_tensor on DVE)

Masking correctness vs reference: reference fills masked logits with -30
and re-zeroes attn post-softmax; its denominator keeps exp(-30 - max)
~1e-13 contributions which are below fp32 resolution of the sum. We use
exp(-100) = 0 and drop fully-masked key tiles entirely.
"""

import math

import numpy as np

import concourse.bass as bass
import concourse.mybir as mybir
import concourse.tile as tile
from concourse import bacc

N_CORES = 8
B = 32
G = 1024
D = 256
BPC = B // N_CORES
TOK = BPC * G
NORM = 1.0 / math.sqrt(D)
MASK_BIAS = -100.0
KD = G // 128                # 8 query tiles per batch
NKT = 5                      # key tiles computed (after compaction)
DT_CH = D // 128             # 2 chunks of the feature dim

F32 = mybir.dt.float32
I32 = mybir.dt.int32
BF16 = mybir.dt.bfloat16


def build_program(nkt: int = NKT, bpc: int = BPC, reps: int = 1,
                  enable_asserts: bool = False):
    assert 1 <= nkt <= KD
    nc = bacc.Bacc("TRN2", target_bir_lowering=False, debug=False,
                   enable_asserts=enable_asserts)

    tok = bpc * G
    data_d = nc.dram_tensor("data", [tok, D], F32, kind="ExternalInput").ap()
    mask_d = nc.dram_tensor("mask", [bpc, G], I32, kind="ExternalInput").ap()
    wq_d = nc.dram_tensor("w_query", [D, D], F32, kind="ExternalInput").ap()
    wk_d = nc.dram_tensor("w_key", [D, D], F32, kind="ExternalInput").ap()
    wv_d = nc.dram_tensor("w_val", [D, D], F32, kind="ExternalInput").ap()
    wo_d = nc.dram_tensor("w_out", [D, D], F32, kind="ExternalInput").ap()
    b_d = nc.dram_tensor("b_out", [D], F32, kind="ExternalInput").ap()
    out_d = nc.dram_tensor("out", [tok, D], F32, kind="ExternalOutput").ap()

    from contextlib import ExitStack
    with tile.TileContext(nc) as tc, ExitStack() as ctx:
        _body(ctx, tc, out_d, data_d, mask_d, wq_d, wk_d, wv_d, wo_d, b_d,
              nkt, bpc, reps)

    nc.compile()
    return nc


def _body(ctx, tc, out_d, data_d, mask_d, wq_d, wk_d, wv_d, wo_d, b_d,
          nkt, bpc, reps):
    nc = tc.nc
    from concourse.masks import make_identity

    const = ctx.enter_context(tc.tile_pool(name="const", bufs=1))
    wpool = ctx.enter_context(tc.tile_pool(name="wpool", bufs=1))
    dnat_p = ctx.enter_context(tc.tile_pool(name="dnat", bufs=10))
    dn16_p = ctx.enter_context(tc.tile_pool(name="dn16", bufs=18))
    dT_p = ctx.enter_context(tc.tile_pool(name="dT", bufs=3))
    qt_p = ctx.enter_context(tc.tile_pool(name="qt", bufs=3))
    pt_p = ctx.enter_context(tc.tile_pool(name="pt", bufs=2))
    ht_p = ctx.enter_context(tc.tile_pool(name="ht", bufs=2))
    out_p = ctx.enter_context(tc.tile_pool(name="outp", bufs=8))
    misc_p = ctx.enter_context(tc.tile_pool(name="misc", bufs=3))

    # PSUM banks: ps_t 1 + ps_q 2 + ps_s 2 + ps_acc 2 + ps_l 1 = 8
    ps_t = ctx.enter_context(tc.tile_pool(name="ps_t", bufs=1, space="PSUM"))
    ps_q = ctx.enter_context(tc.tile_pool(name="ps_q", bufs=1, space="PSUM"))
    ps_s = ctx.enter_context(tc.tile_pool(name="ps_s", bufs=1, space="PSUM"))
    ps_acc = ctx.enter_context(tc.tile_pool(name="ps_acc", bufs=2, space="PSUM"))
    ps_l = ctx.enter_context(tc.tile_pool(name="ps_l", bufs=1, space="PSUM"))

    # ---- constants ----------------------------------------------------------
    ident_f = const.tile([128, 128], F32, tag="identf")
    make_identity(nc, ident_f)
    ident = const.tile([128, 128], BF16, tag="ident")
    nc.vector.tensor_copy(ident, ident_f)

    ones_f32 = const.tile([128, 1], F32, tag="ones_f32")
    nc.vector.memset(ones_f32, 1.0)
    ones = const.tile([128, 1], BF16, tag="ones")
    nc.vector.tensor_copy(ones, ones_f32)

    # warm the exp table set before the first attention tile
    act_warm = const.tile([128, 1], F32, tag="act_warm")
    nc.scalar.activation(out=act_warm, in_=ones_f32,
                         func=mybir.ActivationFunctionType.Exp)

    bias_rep = const.tile([128, D], F32, tag="bias_rep")
    b_bcast = bass.AP(tensor=b_d.tensor, offset=b_d.offset,
                      ap=[[0, 128]] + list(b_d.ap))
    nc.gpsimd.dma_start(out=bias_rep, in_=b_bcast)

    # ---- weight prep: NT = bf16(Wq^T Wk), PTO = bf16(Wv^T Wo^T) -------------
    wnat = {}
    for name, w_d in (("q", wq_d), ("k", wk_d), ("v", wv_d), ("o", wo_d)):
        ts = []
        for r in range(DT_CH):
            t = wpool.tile([128, D], F32, tag=f"wnat_{name}{r}",
                           name=f"wnat_{name}{r}")
            nc.sync.dma_start(out=t, in_=w_d[r * 128:(r + 1) * 128, :])
            ts.append(t)
        wnat[name] = ts

    woT = []
    for c in range(DT_CH):
        wt_c = wpool.tile([128, D], F32, tag=f"woT{c}", name=f"woT{c}")
        for r in range(DT_CH):
            ps = ps_q.tile([128, 1024], F32, tag="ps_q", name=f"psw{c}{r}")
            nc.tensor.transpose(ps[:, :128],
                                wnat["o"][r][:, c * 128:(c + 1) * 128], ident_f)
            nc.scalar.copy(wt_c[:, r * 128:(r + 1) * 128], ps[:, :128])
        woT.append(wt_c)

    nt = []
    for jt in range(DT_CH):
        ps = ps_q.tile([128, 1024], F32, tag="ps_q", name=f"psnt{jt}")
        for dc in range(DT_CH):
            nc.tensor.matmul(
                ps[:, :D],
                wnat["q"][dc][:, jt * 128:(jt + 1) * 128],
                wnat["k"][dc],
                start=(dc == 0), stop=(dc == DT_CH - 1))
        t = wpool.tile([128, D], BF16, tag=f"nt{jt}", name=f"nt{jt}")
        nc.scalar.copy(t, ps[:, :D])
        nt.append(t)

    pto = []
    for dtile in range(DT_CH):
        ps = ps_q.tile([128, 1024], F32, tag="ps_q", name=f"pspt{dtile}")
        for mc in range(DT_CH):
            nc.tensor.matmul(
                ps[:, :D],
                wnat["v"][mc][:, dtile * 128:(dtile + 1) * 128],
                woT[mc],
                start=(mc == 0), stop=(mc == DT_CH - 1))
        t = wpool.tile([128, D], BF16, tag=f"pto{dtile}", name=f"pto{dtile}")
        nc.scalar.copy(t, ps[:, :D])
        pto.append(t)

    state = {}
    _seq = [0]   # unique name suffix: the wrap re-emits stage A for batch 0

    def a_load(b):
        """Mask prep + data DMA + bf16 casts."""
        _seq[0] += 1
        u = f"{b}_{_seq[0]}"
        row0 = b * G
        mb8 = misc_p.tile([nkt, 128], I32, tag="mb8", name=f"mb8_{u}")
        nc.sync.dma_start(
            out=mb8,
            in_=mask_d[b][:nkt * 128].rearrange("(j f) -> j f", j=nkt))
        mbf = misc_p.tile([nkt, 128], F32, tag="mbf", name=f"mbf_{u}")
        nc.vector.tensor_scalar_mul(mbf, mb8, MASK_BIAS)
        ps_mb = ps_q.tile([128, 1024], F32, tag="ps_q", name=f"psmb_{u}")
        nc.tensor.transpose(ps_mb[:, :nkt], mbf, ident_f[:nkt, :nkt])
        mbT = misc_p.tile([128, nkt], F32, tag="mbT", name=f"mbT_{u}")
        nc.vector.tensor_copy(mbT, ps_mb[:, :nkt])

        dn16 = []
        for t in range(KD):
            dn = dnat_p.tile([128, D], F32, tag="dnat", name=f"dn_{u}_{t}")
            (nc.sync if t % 2 == 0 else nc.gpsimd).dma_start(
                out=dn, in_=data_d[row0 + t * 128:row0 + (t + 1) * 128, :])
            d16 = dn16_p.tile([128, D], BF16, tag="dn16", name=f"dn16_{u}_{t}")
            nc.vector.tensor_copy(d16, dn)
            dn16.append(d16)
        state[b] = {"V": dn16, "mbT": mbT, "u": u}

    def a_transpose(b):
        """PE transposes dn16 -> dT (bf16)."""
        st = state[b]
        u = st["u"]
        dT = []
        for c in range(DT_CH):
            dc = dT_p.tile([128, G], BF16, tag=f"dT{c}", name=f"dT_{u}_{c}")
            pst = ps_t.tile([128, 1024], BF16, tag="ps_t", name=f"psdt_{u}_{c}")
            for g in range(KD):
                nc.tensor.transpose(pst[:, g * 128:(g + 1) * 128],
                                    st["V"][g][:, c * 128:(c + 1) * 128], ident)
            nc.vector.tensor_copy(dc, pst)
            dT.append(dc)
        st["dT"] = dT

    def a_qt(b):
        """QT = NT @ dT."""
        st = state[b]
        u = st["u"]
        QT = []
        for dt_i in range(DT_CH):
            psq = ps_q.tile([128, 1024], F32, tag="ps_q", name=f"psq_{u}_{dt_i}")
            for h in range(2):
                for ic in range(DT_CH):
                    nc.tensor.matmul(
                        psq[:, h * 512:(h + 1) * 512],
                        nt[ic][:, dt_i * 128:(dt_i + 1) * 128],
                        st["dT"][ic][:, h * 512:(h + 1) * 512],
                        start=(ic == 0), stop=(ic == DT_CH - 1))
            dst = qt_p.tile([128, G], BF16, tag=f"qT{dt_i}",
                            name=f"qT_{u}_{dt_i}")
            nc.scalar.copy(dst, psq)
            QT.append(dst)
        st["QT"] = QT

    def b_open(b):
        st = state[b]
        st["pt"] = pt_p.tile([128, nkt * G], BF16, tag="pt", name=f"pt_{b}")
        st["HT"] = [ht_p.tile([128, G], BF16, tag=f"hT{i}", name=f"hT_{b}_{i}")
                    for i in range(DT_CH)]
        st["lrow"] = misc_p.tile([1, G], F32, tag="l_row", name=f"lrow_{b}")

    def emit_s(b, kt_i):
        st = state[b]
        pss = ps_s.tile([128, 1024], F32, tag="ps_s", name=f"pss_{b}_{kt_i}")
        for h in range(2):
            for ic in range(DT_CH):
                nc.tensor.matmul(
                    pss[:, h * 512:(h + 1) * 512],
                    st["dT"][ic][:, kt_i * 128:(kt_i + 1) * 128],
                    st["QT"][ic][:, h * 512:(h + 1) * 512],
                    start=(ic == 0), stop=(ic == DT_CH - 1))
        nc.scalar.activation(
            out=st["pt"][:, kt_i * G:(kt_i + 1) * G], in_=pss,
            func=mybir.ActivationFunctionType.Exp,
            bias=st["mbT"][:, kt_i:kt_i + 1], scale=NORM)

    def pv_pass(b, h):
        st = state[b]
        psH = [ps_acc.tile([128, 512], F32, tag="ps_acc",
                           name=f"psH_{b}_{h}_{i}") for i in range(DT_CH)]
        def emit_pv(kt_i):
            for dt_i in range(DT_CH):
                nc.tensor.matmul(
                    psH[dt_i],
                    st["V"][kt_i][:, dt_i * 128:(dt_i + 1) * 128],
                    st["pt"][:, kt_i * G + h * 512:kt_i * G + (h + 1) * 512],
                    start=(kt_i == 0), stop=(kt_i == nkt - 1))
        return psH, emit_pv

    # l is summed in two levels: DVE pair-sums of PT tiles (bf16 2x mode),
    # then a short ones-matmul chain over pairs + leftover tiles.
    l_pairs = [(2 * i, 2 * i + 1) for i in range(nkt // 2)]
    l_rest = list(range(2 * (nkt // 2), nkt))

    def emit_lsums(b):
        st = state[b]
        sums = []
        for pi, (i0, i1) in enumerate(l_pairs):
            sm = misc_p.tile([128, G], BF16, tag=f"lsum{pi}",
                             name=f"lsum_{b}_{pi}")
            nc.vector.tensor_tensor(
                out=sm, in0=st["pt"][:, i0 * G:(i0 + 1) * G],
                in1=st["pt"][:, i1 * G:(i1 + 1) * G],
                op=mybir.AluOpType.add)
            sums.append(sm)
        st["lsums"] = sums

    def emit_l(b, h, psl):
        st = state[b]
        terms = ([(sm, 0) for sm in st["lsums"]]
                 + [(st["pt"], kt_i * G) for kt_i in l_rest])
        for i, (src, off) in enumerate(terms):
            nc.tensor.matmul(
                psl, ones, src[:, off + h * 512:off + (h + 1) * 512],
                start=(i == 0), stop=(i == len(terms) - 1))

    def b_inv(b):
        st = state[b]
        ps_inv = ps_q.tile([128, 1024], F32, tag="ps_q", name=f"psinv_{b}")
        for j in range(KD):
            nc.tensor.transpose(
                ps_inv[:, j:j + 1], st["lrow"][:, j * 128:(j + 1) * 128],
                ident_f[:1, :1])
        invl = misc_p.tile([128, KD], F32, tag="invl", name=f"invl_{b}")
        nc.vector.reciprocal(invl, ps_inv[:, :KD])
        st["invl"] = invl

    def stage_c(b):
        st = state[b]
        HT, invl = st["HT"], st["invl"]
        row0 = b * G

        for p_i in range(KD // 2):
            ps = ps_acc.tile([128, 512], F32, tag="ps_acc",
                             name=f"psf_{b}_{p_i}")
            for j in range(2):
                qt_i = p_i * 2 + j
                for dt_i in range(DT_CH):
                    nc.tensor.matmul(
                        ps[:, j * D:(j + 1) * D],
                        HT[dt_i][:, qt_i * 128:(qt_i + 1) * 128],
                        pto[dt_i],
                        start=(dt_i == 0), stop=(dt_i == DT_CH - 1))
            for j in range(2):
                qt_i = p_i * 2 + j
                ot = out_p.tile([128, D], F32, tag="outp", name=f"ot_{b}_{qt_i}")
                nc.vector.scalar_tensor_tensor(
                    out=ot, in0=ps[:, j * D:(j + 1) * D],
                    scalar=invl[:, qt_i:qt_i + 1], in1=bias_rep,
                    op0=mybir.AluOpType.mult, op1=mybir.AluOpType.add)
                (nc.sync if qt_i % 2 == 0 else nc.gpsimd).dma_start(
                    out=out_d[row0 + qt_i * 128:row0 + (qt_i + 1) * 128, :],
                    in_=ot)
        del state[b]

    # software pipeline: next-batch stage-A PE work is emitted inside the
    # two windows where this batch's PE stream would stall (the last exp's
    # latency, and the HT-drain latency before the final projection).
    # Under a reps-loop the pipeline wraps: batch 0's stage A for the next
    # iteration is emitted in batch bpc-1's windows, so the hardware loop
    # body is a true steady state (the prologue stage A runs once, outside).
    wrap = reps > 1

    def batch_body(b):
        nxt = b + 1 if b + 1 < bpc else (0 if wrap else None)
        b_open(b)
        if nxt is not None:
            a_load(nxt)           # DMA + DVE casts run under the S/PV head
        psH0, emit_pv0 = pv_pass(b, 0)
        emit_s(b, 0)
        for kt_i in range(1, nkt):
            emit_s(b, kt_i)
            emit_pv0(kt_i - 1)
        emit_lsums(b)
        if nxt is not None:
            a_transpose(nxt)      # fills the exp(nkt-1) latency window
        emit_pv0(nkt - 1)
        psl0 = ps_l.tile([1, 512], F32, tag="ps_l", name=f"psl_{b}_0")
        emit_l(b, 0, psl0)
        for dt_i in range(DT_CH):
            nc.vector.tensor_copy(state[b]["HT"][dt_i][:, 0:512], psH0[dt_i])

        psH1, emit_pv1 = pv_pass(b, 1)
        for kt_i in range(nkt):
            emit_pv1(kt_i)
        nc.scalar.copy(state[b]["lrow"][:, 0:512], psl0)
        psl1 = ps_l.tile([1, 512], F32, tag="ps_l", name=f"psl_{b}_1")
        emit_l(b, 1, psl1)
        for dt_i in range(DT_CH):
            nc.scalar.copy(state[b]["HT"][dt_i][:, 512:1024], psH1[dt_i])
        nc.scalar.copy(state[b]["lrow"][:, 512:1024], psl1)

        if nxt is not None:
            a_qt(nxt)             # fills the HT-drain latency window
        b_inv(b)
        stage_c(b)

    a_load(0)
    a_transpose(0)
    a_qt(0)

    if reps > 1:
        loop_cm = tc.For_i(0, reps, 1)
        loop_cm.__enter__()

    for b in range(bpc):
        batch_body(b)

    if reps > 1:
        loop_cm.__exit__(None, None, None)


# ---------------------------------------------------------------------------
# Host side: compaction + a cached jax.jit(shard_map) runner over the 8 cores.
def compact(data, mask):
    """Per-batch stable-sort of rows so unmasked keys come first."""
    nb = mask.shape[0]
    datac = np.empty_like(data)
    maskc = np.empty_like(mask)
    perms = np.empty((nb, G), np.int64)
    for b in range(nb):
        p = np.argsort(mask[b], kind="stable")
        datac[b * G:(b + 1) * G] = data[b * G:(b + 1) * G][p]
        maskc[b] = mask[b][p]
        perms[b] = p
    return datac, maskc, perms


_RUNNER_CACHE = {}


def _make_runner(nkt):
    import jax
    from jax.experimental.shard_map import shard_map
    from jax.sharding import Mesh, NamedSharding, PartitionSpec

    from concourse.bass2jax import (
        _bass_exec_p,
        install_neuronx_cc_hook,
        partition_id_tensor,
    )

    nc = build_program(nkt)
    install_neuronx_cc_hook()
    assert nc.dbg_addr is None
    partition_name = (nc.partition_id_tensor.name
                      if nc.partition_id_tensor else None)

    in_names, out_names, out_avals, zero_outs = [], [], [], []
    for alloc in nc.m.functions[0].allocations:
        if not isinstance(alloc, mybir.MemoryLocationSet):
            continue
        name = alloc.memorylocations[0].name
        if alloc.kind == "ExternalInput":
            if name != partition_name:
                in_names.append(name)
        elif alloc.kind == "ExternalOutput":
            shape = tuple(alloc.tensor_shape)
            dtype = mybir.dt.np(alloc.dtype)
            out_names.append(name)
            out_avals.append(jax.core.ShapedArray(shape, dtype))
            zero_outs.append(np.zeros((N_CORES * shape[0],) + shape[1:], dtype))
    n_params = len(in_names)
    all_in_names = list(in_names) + list(out_names)
    if partition_name is not None:
        all_in_names.append(partition_name)

    def _body_fn(*args):
        operands = list(args)
        if partition_name is not None:
            operands.append(partition_id_tensor())
        outs = _bass_exec_p.bind(
            *operands,
            out_avals=tuple(out_avals),
            in_names=tuple(all_in_names),
            out_names=tuple(out_names),
            lowering_input_output_aliases=(),
            sim_require_finite=False,
            sim_require_nnan=False,
            nc=nc,
        )
        return tuple(outs)

    devices = jax.devices()[:N_CORES]
    mesh = Mesh(np.asarray(devices), ("core",))
    in_specs = (PartitionSpec("core"),) * (n_params + len(out_names))
    out_specs = (PartitionSpec("core"),) * len(out_names)
    sharded = jax.jit(
        shard_map(_body_fn, mesh=mesh, in_specs=in_specs, out_specs=out_specs,
                  check_rep=False),
        keep_unused=True,
    )
    sharding = NamedSharding(mesh, PartitionSpec("core"))
    dev_zeros = [jax.device_put(z, sharding) for z in zero_outs]
    return {
        "nc": nc, "fn": sharded, "in_names": in_names,
        "out_names": out_names, "sharding": sharding, "dev_zeros": dev_zeros,
    }


def get_runner(nkt=NKT):
    if nkt not in _RUNNER_CACHE:
        _RUNNER_CACHE[nkt] = _make_runner(nkt)
    return _RUNNER_CACHE[nkt]


def _concat_inputs(data, mask, wq, wk, wv, wo, b):
    """Per-core shards concatenated on axis 0, keyed by dram tensor name."""
    return {
        "data": data,
        "mask": mask,
        "w_query": np.concatenate([wq] * N_CORES, axis=0),
        "w_key": np.concatenate([wk] * N_CORES, axis=0),
        "w_val": np.concatenate([wv] * N_CORES, axis=0),
        "w_out": np.concatenate([wo] * N_CORES, axis=0),
        "b_out": np.concatenate([b] * N_CORES, axis=0),
    }


def kernel(data, mask, graph_size, evaluate, W_query, W_key, W_val, W_out,
           b_out, **_ignored):
    data = np.ascontiguousarray(np.asarray(data, dtype=np.float32))
    mask = np.ascontiguousarray(np.asarray(mask, dtype=np.int32))
    wq = np.ascontiguousarray(np.asarray(W_query, dtype=np.float32))
    wk = np.ascontiguousarray(np.asarray(W_key, dtype=np.float32))
    wv = np.ascontiguousarray(np.asarray(W_val, dtype=np.float32))
    wo = np.ascontiguousarray(np.asarray(W_out, dtype=np.float32))
    b = np.ascontiguousarray(np.asarray(b_out, dtype=np.float32))

    datac, maskc, perms = compact(data, mask)
    nk_max = int((G - maskc.sum(axis=1)).max())
    nkt = max(NKT, -(-nk_max // 128))   # ceil; >=NKT so the cached program wins

    r = get_runner(nkt)
    cat = _concat_inputs(datac, maskc, wq, wk, wv, wo, b)
    args = [cat[n] for n in r["in_names"]] + list(r["dev_zeros"])
    outs = r["fn"](*args)
    outc = np.asarray(outs[r["out_names"].index("out")])

    out = np.empty_like(outc)
    rows = (perms + (np.arange(B)[:, None] * G)).reshape(-1)
    out[rows] = outc
    return out
